# revision 8
# baseline (speedup 1.0000x reference)
"""ConvLSTM seq2seq (B=8,T=10,C=1,H=W=64,HID=64, 2 enc + 2 dec cells, pred_len=10)
on 8 Trainium2 NeuronCores, data-parallel over batch (1 sample per core).

v3: conv3x3 as fp8e4m3 DoubleRow tap-pair matmuls; each padded image stored
twice in one SBUF tile (plane1 = plane0 shifted left 2 cols) so a [128, 2, N]
rhs streams two taps per matmul: 9 taps = 1 f16 identity inject (opens the
PSUM group with the peephole, computed one cell AHEAD so PE never waits) +
4 DoubleRow fp8 matmuls + 1 plain fp8 matmul. Spatial tiling = full-width
padded-row strips (7 rows x 66 = 462 cols; junk at x=0,65 discarded), strips
PAIRED into [128, 1024] two-bank PSUM tiles so every ACT/DVE op covers two
strips (halves per-op fixed overheads). Gate math: ACT sigmoid/tanh (f16),
DVE P=[i*g|f*c], c_new = P_lo+P_hi, h = o*tanh(c) written fp8 into the image;
plane dups via SBUF-SBUF DMA. Emission is software-pipelined with a one-pair
lag so the ACT and DVE streams never ping-pong within a strip.
"""
import sys
import os

sys.path.insert(0, '/opt/trn_rl_repo')
import numpy as np
import ml_dtypes

B, T, C, H, W = 8, 10, 1, 64, 64
HID = 64
F = 10                  # prediction_len (hardcoded; asserted in kernel())
PW = 66                 # padded row width
NPAD = 66 * 66          # 4356
NPA = 4368              # plane stride (16-aligned)
GUARD = 16              # left guard so tap (0,0) at strip 0 reads zeros
BRS = 9280              # fp8 bias region (2 groups x 4640, strip stride 462)
P0 = 9308               # plane0 base within image tile (b region + align pad)
ILEN = 23024            # image tile length: fits a+2*S rearrange windows
NSP = H * W             # 4096
NSTRIP = 10             # 9 strips of 7 rows + 1 strip of 1 row
SROWS = [7] * 9 + [1]
NPAIR = 5               # strips paired (2 per PSUM tile / elementwise op)
WST = 512               # f16 strip stride in wc/b/c2/T2/scratch layouts
CW = NSTRIP * WST       # 5120 elements per f16 state row
PAIRS = [((0, 0), (0, 2)), ((0, 1), (1, 1)), ((1, 0), (1, 2)), ((2, 0), (2, 2))]
SINGLE = (2, 1)

_CACHE = {}


def _build_nc():
    import concourse.tile as tile
    from concourse import bacc, mybir

    f32 = mybir.dt.float32
    f16 = mybir.dt.float16
    f8 = mybir.dt.float8e4
    AF = mybir.ActivationFunctionType
    DR = mybir.MatmulPerfMode.DoubleRow

    nc = bacc.Bacc("TRN2", target_bir_lowering=False, debug=False, num_devices=8)

    x_d = nc.dram_tensor("x", (T, 2 * NPA), f8, kind="ExternalInput").ap()
    wdr_d = nc.dram_tensor("wdr", (128, 8 * 1280), f8, kind="ExternalInput").ap()
    wce_d = nc.dram_tensor("wc_enc", (128, 4 * CW), f16, kind="ExternalInput").ap()
    wcd_d = nc.dram_tensor("wc_dec", (128, 4 * CW), f16, kind="ExternalInput").ap()
    b8_d = nc.dram_tensor("b8", (128, 4 * BRS), f8, kind="ExternalInput").ap()
    id_d = nc.dram_tensor("ident", (128, 128), f16, kind="ExternalInput").ap()
    fin_d = nc.dram_tensor("fin", (128, 144), f8, kind="ExternalInput").ap()
    finb_d = nc.dram_tensor("finb", (1, 1), f32, kind="ExternalInput").ap()
    y_d = nc.dram_tensor("y", (F, NSP), f32, kind="ExternalOutput").ap()

    def pN(p):  # elementwise span of pair p (covers both strips + psum gap)
        return 974 if p < 4 else 578

    with tile.TileContext(nc) as tc:
        with tc.tile_pool(name="persist", bufs=1) as pp, \
             tc.tile_pool(name="wcb", bufs=1) as wp, \
             tc.tile_pool(name="sc", bufs=2) as sc, \
             tc.tile_pool(name="ps", bufs=2, space="PSUM") as ps:

            I0 = pp.tile([128, ILEN], f8, name="I0", tag="I0")
            I1 = pp.tile([128, ILEN], f8, name="I1", tag="I1")
            C2 = [pp.tile([128, CW], f16, name=f"C2_{i}", tag=f"C2_{i}")
                  for i in range(2)]
            T2 = [[pp.tile([128, CW], f16, name=f"T2_{p}_{g}", tag=f"T2_{p}_{g}")
                   for g in (0, 1)] for p in (0, 1)]
            wdr_t = pp.tile([128, 8 * 1280], f8)
            id_t = pp.tile([128, 128], f16)
            fin_t = pp.tile([128, 144], f8)
            fb_t = pp.tile([1, 1], f32)

            nc.sync.dma_start(wdr_t[:], wdr_d[:])
            nc.sync.dma_start(id_t[:], id_d[:])
            nc.sync.dma_start(fin_t[:], fin_d[:])
            nc.sync.dma_start(fb_t[:], finb_d[:])
            # zero-init images (incl guard + pads), cell states, t2 buffers
            nc.vector.memset(I0[:].bitcast(f32), 0.0)
            nc.vector.memset(I1[:].bitcast(f32), 0.0)
            nc.vector.memset(C2[0][:].bitcast(f32), 0.0)
            nc.vector.memset(C2[1][:].bitcast(f32), 0.0)
            for pr in (0, 1):
                for g in (0, 1):
                    nc.vector.memset(T2[pr][g][:].bitcast(f32), 0.0)

            wc_t = wp.tile([128, 4 * CW], f16, tag="wc")
            nc.sync.dma_start(I0[:, GUARD:GUARD + BRS], b8_d[:, 0:BRS])
            nc.sync.dma_start(I1[:, GUARD:GUARD + BRS], b8_d[:, BRS:2 * BRS])
            nc.sync.dma_start(wc_t[:], wce_d[:])

            def pair_rhs(img_t, s, tA, tB, N, plo=0, phi=128):
                # [K, 2, N] AP: plane0 at tap A, plane1 (pre-shifted by -2) at tap B
                a = P0 + (7 * s + tA[0]) * PW + tA[1] - 1
                b_off = P0 + NPA + (7 * s + tB[0]) * PW + tB[1] - 1 - 2
                S = b_off - a
                v = img_t[plo:phi, a:a + 2 * S].rearrange("k (two n) -> k two n", two=2)
                return v[:, :, 0:N]

            def tap_rhs(img_t, s, t, N, plo=0, phi=128):
                a = P0 + (7 * s + t[0]) * PW + t[1] - 1
                return img_t[plo:phi, a:a + N]

            def injb_rhs(img_t, s, g, N):
                # [K, 2, N]: half0 = bias region (identity weights), half1 =
                # plane0 at the single tap (2,1)
                X0 = GUARD + 4640 * g + 462 * s
                S = (P0 + 462 * s + 132) - X0
                v = img_t[:, X0:X0 + 2 * S].rearrange("k (two n) -> k two n", two=2)
                return v[:, :, 0:N]

            def emit_t2(wcb_half, c2, t2_out, p):
                # t2 = wc (.) c over pair p, both gate-pair groups (f16)
                N = pN(p)
                sl = slice(1024 * p, 1024 * p + N)
                for g in (0, 1):
                    w0 = wcb_half * 2 * CW + g * CW + 1024 * p
                    nc.vector.tensor_mul(t2_out[g][:, sl], wc_t[:, w0:w0 + N],
                                         c2[:, sl])

            def cell_step(wci, img_t, c2, t2_in, hp_img, dups, t2_next=None):
                """One ConvLSTM cell update, software-pipelined over strip pairs.
                wci: conv-weight cell index 0..3 (enc0,enc1,dec0,dec1) into wdr_t.
                img_t: input image tile (conv rhs). c2: [g|c] / [c|c] state.
                t2_in: [tileA, tileB] peephole for this cell (from prev cell).
                hp_img: tile getting the primary h write (partitions 64:128).
                dups: list of (img_tile, part_lo, plane) DMA dup targets.
                t2_next: None or (wcb_half, c2, t2_out) for the NEXT cell.
                """
                wofs = wci * 2560  # 2 groups x 1280 per cell
                state = {}

                def front(p):
                    N = pN(p)
                    sl = slice(1024 * p, 1024 * p + N)
                    pAB = []
                    for g in (0, 1):
                        pg = ps.tile([128, 1024], f32, tag=f"p{g}", bufs=2)
                        gofs = wofs + g * 1280
                        for half in (0, 1):
                            s = 2 * p + half
                            if s >= NSTRIP:
                                break
                            Ns = SROWS[s] * PW
                            o = 512 * half
                            ninj = 512 if half == 0 else Ns
                            nc.tensor.matmul(pg[:, o:o + ninj], id_t[:],
                                             t2_in[g][:, 512 * s:512 * s + ninj],
                                             start=True, stop=False)
                            for j, (tA, tB) in enumerate(PAIRS):
                                lhs = wdr_t[:, gofs + j * 256: gofs + j * 256 + 256] \
                                    .rearrange("k (two m) -> k two m", two=2)
                                nc.tensor.matmul(pg[:, o:o + Ns], lhs,
                                                 pair_rhs(img_t, s, tA, tB, Ns),
                                                 start=False, stop=False, perf_mode=DR)
                            lhsb = wdr_t[:, gofs + 1024: gofs + 1280] \
                                .rearrange("k (two m) -> k two m", two=2)
                            nc.tensor.matmul(pg[:, o:o + Ns], lhsb,
                                             injb_rhs(img_t, s, g, Ns),
                                             start=False, stop=True, perf_mode=DR)
                        pAB.append(pg)
                    pA, pB = pAB
                    # gates: pA = [i|f] -> sigmoid; pB = [g|o] -> tanh/sigmoid
                    sigA = sc.tile([128, 1024], f16, tag="sigA", bufs=2)
                    nc.scalar.activation(sigA[:, 0:N], pA[:, 0:N], AF.Sigmoid)
                    nc.scalar.activation(c2[0:64, sl], pB[0:64, 0:N], AF.Tanh)
                    so = sc.tile([128, 1024], f16, tag="so", bufs=3)
                    nc.scalar.activation(so[64:128, 0:N], pB[64:128, 0:N], AF.Sigmoid)
                    # P = [i*g | f*c]; c_new = P_lo + P_hi -> both halves of c2
                    P = sc.tile([128, 1024], f16, tag="P", bufs=3)
                    nc.vector.tensor_mul(P[:, 0:N], sigA[:, 0:N], c2[:, sl])
                    # TensorTensor inputs must share a base partition: realign
                    # P_hi to base 0 with a 1-input copy, then sum the halves
                    Pc = sc.tile([64, 1024], f16, tag="Pc", bufs=2)
                    nc.vector.tensor_copy(Pc[:, 0:N], P[64:128, 0:N])
                    nc.vector.tensor_add(c2[64:128, sl], P[0:64, 0:N], Pc[:, 0:N])
                    nc.vector.tensor_copy(c2[0:64, sl], c2[64:128, sl])
                    state[p] = so

                def back(p):
                    N = pN(p)
                    sl = slice(1024 * p, 1024 * p + N)
                    so = state.pop(p)
                    th = sc.tile([128, 1024], f16, tag="th", bufs=2)
                    nc.scalar.activation(th[64:128, 0:N], c2[64:128, sl], AF.Tanh)
                    for half in (0, 1):
                        s = 2 * p + half
                        if s >= NSTRIP:
                            break
                        nr = SROWS[s]
                        o = 512 * half
                        w0 = P0 + (7 * s + 1) * PW + 1
                        hv = hp_img[64:128, w0:w0 + nr * PW] \
                            .rearrange("k (r c) -> k r c", c=PW)[:, :, 0:64]
                        so_v = so[64:128, o:o + nr * PW] \
                            .rearrange("p (r c) -> p r c", c=PW)[:, :, 1:65]
                        th_v = th[64:128, o:o + nr * PW] \
                            .rearrange("p (r c) -> p r c", c=PW)[:, :, 1:65]
                        nc.gpsimd.tensor_mul(hv, so_v, th_v)
                        # plane1 of self (cols shifted -2), then both planes of
                        # the other image in one 2-plane DMA
                        sv1 = hp_img[64:128, w0 + NPA - 2:w0 + NPA - 2 + nr * PW] \
                            .rearrange("k (r c) -> k r c", c=PW)[:, :, 0:64]
                        nc.gpsimd.dma_start(sv1, hv)
                        if dups:
                            dimg, plo = dups[0]
                            for off in (w0, w0 + NPA - 2):
                                dv = dimg[plo:plo + 64, off:off + nr * PW] \
                                    .rearrange("k (r c) -> k r c", c=PW)[:, :, 0:64]
                                sv = hp_img[64:128, off:off + nr * PW] \
                                    .rearrange("k (r c) -> k r c", c=PW)[:, :, 0:64]
                                nc.sync.dma_start(dv, sv)
                    if t2_next is not None:
                        emit_t2(*t2_next, p)

                for p in range(NPAIR + 1):
                    if p < NPAIR:
                        front(p)
                    if p >= 1:
                        back(p - 1)

            def final_conv(f):
                for p in range(NPAIR):
                    N = pN(p)
                    pf = ps.tile([16, 1024], f32, tag="p0", bufs=2)
                    for half in (0, 1):
                        s = 2 * p + half
                        if s >= NSTRIP:
                            break
                        Ns = SROWS[s] * PW
                        o = 512 * half
                        # single tap first with N=512 so the psum gap is
                        # defined; tap pairs as DoubleRow with M padded to 16
                        # (M=1 weights violate the dual-fp8 LDWEIGHTS step
                        # restriction; only psum row 0 is read)
                        nc.tensor.matmul(pf[0:16, o:o + 512], fin_t[64:128, 128:144],
                                         tap_rhs(I1, s, SINGLE, 512, plo=64, phi=128),
                                         start=True, stop=False)
                        for j, (tA, tB) in enumerate(PAIRS):
                            lhs = fin_t[64:128, 32 * j: 32 * j + 32] \
                                .rearrange("k (two m) -> k two m", two=2)
                            nc.tensor.matmul(pf[0:16, o:o + Ns], lhs,
                                             pair_rhs(I1, s, tA, tB, Ns, plo=64, phi=128),
                                             start=False, stop=(j == 3), perf_mode=DR)
                    yt = sc.tile([1, 1024], f32, tag="yt", bufs=2)
                    nc.scalar.activation(yt[:, 0:N], pf[0:1, 0:N], AF.Sigmoid,
                                         bias=fb_t[0:1, 0:1])
                    for half in (0, 1):
                        s = 2 * p + half
                        if s >= NSTRIP:
                            break
                        nr = SROWS[s]
                        yv = yt[0:1, 512 * half:512 * half + nr * PW] \
                            .rearrange("p (r c) -> p r c", c=PW)[:, :, 1:65]
                        nc.sync.dma_start(
                            y_d[f:f + 1, 448 * s: 448 * s + 64 * nr]
                            .rearrange("p (r c) -> p r c", c=64), yv)

            # ---------------- encoder ----------------
            # (t2 for enc0@t=0 is all-zero: T2 memset; bias comes via [I|b])
            cnt = 0
            for t in range(T):
                nc.sync.dma_start(I0[0:1, P0:P0 + 2 * NPA], x_d[t:t + 1, :])
                cell_step(0, I0, C2[0], T2[cnt % 2], I0, [(I1, 0)],
                          t2_next=(1, C2[1], T2[(cnt + 1) % 2]))
                cnt += 1
                nxt = (0, C2[0], T2[(cnt + 1) % 2]) if t < T - 1 else None
                cell_step(1, I1, C2[1], T2[cnt % 2], I1, [],
                          t2_next=nxt)
                cnt += 1

            # ---------------- transition ----------------
            # s_0 = eh1 : I0[0:64] <- I1[64:128] (both planes); zero h-state
            nc.sync.dma_start(I0[0:64, P0:P0 + 2 * NPA],
                              I1[64:128, P0:P0 + 2 * NPA])
            nc.vector.memset(I0[64:128, P0:P0 + 2 * NPA].bitcast(f32), 0.0)
            nc.vector.memset(I1[:, P0:P0 + 2 * NPA].bitcast(f32), 0.0)
            nc.vector.memset(C2[0][:].bitcast(f32), 0.0)
            nc.vector.memset(C2[1][:].bitcast(f32), 0.0)
            for g in (0, 1):   # t2 for dec0@f=0 must be zero (c2 = 0)
                nc.vector.memset(T2[cnt % 2][g][:].bitcast(f32), 0.0)
            nc.sync.dma_start(I0[:, GUARD:GUARD + BRS], b8_d[:, 2 * BRS:3 * BRS])
            nc.sync.dma_start(I1[:, GUARD:GUARD + BRS], b8_d[:, 3 * BRS:4 * BRS])
            wc_t = wp.tile([128, 4 * CW], f16, tag="wc")
            nc.sync.dma_start(wc_t[:], wcd_d[:])

            # ---------------- decoder ----------------
            for f in range(F):
                cell_step(2, I0, C2[0], T2[cnt % 2], I0, [(I1, 0)],
                          t2_next=(1, C2[1], T2[(cnt + 1) % 2]))
                cnt += 1
                if f > 0:
                    final_conv(f - 1)  # fills the DVE/Pool bubble behind dec0
                nxt = (0, C2[0], T2[(cnt + 1) % 2]) if f < F - 1 else None
                cell_step(3, I1, C2[1], T2[cnt % 2], I1, [(I0, 0)],
                          t2_next=nxt)
                cnt += 1
            final_conv(F - 1)

    nc.compile()
    return nc


def _prep_weights(inputs):
    """Host-side: build per-core DRAM layouts (shared across cores)."""
    f8 = ml_dtypes.float8_e4m3

    def lhsT(Wx, Wh, tap, g):
        # [128 K, 128 M] for one tap / gate-pair group
        dy, dx = tap
        out = np.zeros((128, 128), dtype=np.float32)
        inc = Wx.shape[1]
        W4x = Wx.reshape(4, 64, inc, 3, 3)
        W4h = Wh.reshape(4, 64, 64, 3, 3)
        blkx = np.concatenate([W4x[2 * g], W4x[2 * g + 1]], axis=0)  # [128, inc, 3, 3]
        blkh = np.concatenate([W4h[2 * g], W4h[2 * g + 1]], axis=0)
        out[0:inc, :] = blkx[:, :, dy, dx].T
        out[64:128, :] = blkh[:, :, dy, dx].T
        return out

    def cell_wblocks(Wx, Wh):
        # [128, 2 groups x 5*256] fp8: 4 tap-pair DR blocks + [identity|single]
        I128 = np.eye(128, dtype=np.float32)
        parts = []
        for g in (0, 1):
            for (tA, tB) in PAIRS:
                blk = np.stack([lhsT(Wx, Wh, tA, g), lhsT(Wx, Wh, tB, g)], axis=1)
                parts.append(blk.reshape(128, 256))
            blk = np.stack([I128, lhsT(Wx, Wh, SINGLE, g)], axis=1)
            parts.append(blk.reshape(128, 256))
        return np.concatenate(parts, axis=1)

    def wcb_cell(Pt):
        # Pt: (4, 64, H, W) -> [128, 2, CW] f16 strip layout w/ junk cols
        P4 = np.asarray(Pt, np.float32).reshape(4, 64, 64, 64)
        wide = np.zeros((128, 2, 64, PW), dtype=np.float32)
        for g in (0, 1):
            for half in (0, 1):
                wide[64 * half:64 * half + 64, g, :, 1:65] = P4[2 * g + half]
        out = np.zeros((128, 2, CW), dtype=np.float16)
        for s in range(NSTRIP):
            nr = SROWS[s]
            seg = wide[:, :, 7 * s:7 * s + nr, :].reshape(128, 2, nr * PW)
            out[:, :, WST * s:WST * s + nr * PW] = seg
        return out.reshape(128, 2 * CW)

    g = lambda n: np.asarray(inputs[n], dtype=np.float32)
    wdr = np.concatenate([
        cell_wblocks(g('enc0_Wx'), g('enc0_Wh')),
        cell_wblocks(g('enc1_Wx'), g('enc1_Wh')),
        cell_wblocks(g('dec0_Wx'), g('dec0_Wh')),
        cell_wblocks(g('dec1_Wx'), g('dec1_Wh')),
    ], axis=1).astype(f8)

    wc_enc = np.concatenate([wcb_cell(g('enc0_Wc')), wcb_cell(g('enc1_Wc'))], axis=1)
    wc_dec = np.concatenate([wcb_cell(g('dec0_Wc')), wcb_cell(g('dec1_Wc'))], axis=1)

    def b8_cell(Pt):
        # (4, 64, H, W) -> [128, BRS] fp8, strip stride 462 (junk cols = 0)
        P4 = np.asarray(Pt, np.float32).reshape(4, 64, 64, 64)
        wide = np.zeros((128, 2, 64, PW), dtype=np.float32)
        for gg in (0, 1):
            for half in (0, 1):
                wide[64 * half:64 * half + 64, gg, :, 1:65] = P4[2 * gg + half]
        out = np.zeros((128, 2, 4640), dtype=np.float32)
        for s in range(NSTRIP):
            nr = SROWS[s]
            seg = wide[:, :, 7 * s:7 * s + nr, :].reshape(128, 2, nr * PW)
            out[:, :, 462 * s:462 * s + nr * PW] = seg
        return out.reshape(128, BRS).astype(f8)

    b8 = np.concatenate([b8_cell(g('enc0_b')), b8_cell(g('enc1_b')),
                         b8_cell(g('dec0_b')), b8_cell(g('dec1_b'))], axis=1)

    ident = np.eye(128, dtype=np.float16)

    fin_w = g('fin_w')  # (1, 64, 3, 3)
    fin = np.zeros((128, 144), dtype=np.float32)
    for j, (tA, tB) in enumerate(PAIRS):
        fin[64:128, 32 * j] = fin_w[0, :, tA[0], tA[1]]
        fin[64:128, 32 * j + 16] = fin_w[0, :, tB[0], tB[1]]
    fin[64:128, 128] = fin_w[0, :, SINGLE[0], SINGLE[1]]
    finb = np.asarray(inputs['fin_b'], dtype=np.float32).reshape(1, 1)

    return dict(wdr=wdr, wc_enc=wc_enc, wc_dec=wc_dec, b8=b8,
                ident=ident, fin=fin.astype(f8), finb=finb)


def _prep_x(x_b):
    # x_b: (T, 1, H, W) f32 -> [T, 2*NPA] fp8 (plane0 padded, plane1 shifted -2)
    f8 = ml_dtypes.float8_e4m3
    out = np.zeros((T, 2, NPA), dtype=np.float32)
    pad = np.zeros((T, PW, PW), dtype=np.float32)
    pad[:, 1:65, 1:65] = x_b[:, 0]
    flat = pad.reshape(T, NPAD)
    out[:, 0, :NPAD] = flat
    out[:, 1, :NPAD - 2] = flat[:, 2:]
    return out.reshape(T, 2 * NPA).astype(f8)


def kernel(**inputs):
    from concourse.bass_utils import run_bass_kernel_spmd

    x = np.asarray(inputs['x'], dtype=np.float32)
    assert x.shape == (B, T, C, H, W), x.shape
    assert int(inputs['prediction_len']) == F

    shared = _prep_weights(inputs)
    in_maps = [{**shared, 'x': _prep_x(x[bi])} for bi in range(B)]

    if 'nc' not in _CACHE:
        _CACHE['nc'] = _build_nc()
    nc = _CACHE['nc']

    global _last_in_map
    _last_in_map = in_maps[0]

    trace = os.environ.get('KERNEL_TRACE', '0') == '1'
    res = run_bass_kernel_spmd(nc, in_maps, core_ids=list(range(B)),
                               trace=trace, trace_cores=[0] if trace else None)
    kernel._last_exec_ns = res.exec_time_ns
    kernel._last_trace = res.instructions_and_trace[1] if res.instructions_and_trace else None

    out = np.stack([r['y'].reshape(F, 1, H, W) for r in res.results], axis=0)
    return out.astype(np.float32)
